# revision 24
# baseline (speedup 1.0000x reference)
"""CrossTuckerLayer kernel for 8x Trainium2 NeuronCores (Bass/Tile).

Computes y = einsum('bnvade,ABCDEF,oA,pB,qC,aD,dE,eF->bnvopq', ...)
reshaped to [b, n, v, o*p, q], data-parallel over the 2048 (b,n,v) samples
(256 per core), with the tiny Tucker factors folded host-side into three
small matrices:

  stage A (PE, fp32):  per sample, x_s viewed as [(a,dh)=128, (dlo,e)=128]
      is the stationary operand; one matmul against W1 [(a,dh), (v,D,E)=8]
      contracts a and d_hi, leaving t[(dlo,e), (v,D,E)] with e on partitions.
  stage B (PE, fp32):  8 accumulating matmuls against the block-diagonal
      G2bd [(dlo,e)=128, 72] (core x a2 folded, d_lo matched to v, with the
      rank-8 output replicated into 32-aligned row groups) -> s2T [72, s].
  stage C (PE, bf16 3-limb): s2 is split on-chip into 3 bf16 limbs
      (h + m + l ~ 24 mantissa bits); W_out = u0 (x) u1 (x) u2 is pre-split
      host-side. The six product terms that matter are packed into one
      K=96 matmul per output chunk (scaled duplicate W groups make every
      row meaningful), so stage C runs at bf16 rate (1 cycle/row) with
      fp32-grade accuracy.
"""

import numpy as np
import ml_dtypes

import concourse.bass as bass
import concourse.bacc as bacc
import concourse.mybir as mybir
from concourse.tile import TileContext
from concourse.bass_utils import run_bass_kernel_spmd

F32 = mybir.dt.float32
BF16 = mybir.dt.bfloat16
BF = ml_dtypes.bfloat16

NCORES = 8
S_TOT = 2048          # 4*64*8 samples
S = S_TOT // NCORES   # 256 per core
FIN = 16 * 16 * 64    # 16384
FOUT = 256 * 128      # 32768
S_BLK = 32            # samples per x DMA block
N_BLK = S // S_BLK    # 8
WIN = 128             # samples per stage-C window (out partition dim)
N_WIN = S // WIN      # 2
BLK_PER_WIN = WIN // S_BLK  # 4
ROWS = 96             # limb/W operand rows (12 groups of 8)
YCHUNK = 512          # psum bank (fp32)
YSTAGE = 4096         # cols per y staging tile / output DMA
NC_PER_YSTAGE = YSTAGE // YCHUNK  # 8
N_YSTAGE = FOUT // YSTAGE         # 8 per window


def _host_weights(core, u0, u1, u2, a0, a1, a2):
    """Fold the Tucker factors into the three on-chip matrices (float64)."""
    a0 = a0.astype(np.float64)
    a1 = a1.astype(np.float64)
    a2 = a2.astype(np.float64)

    # W1 [(a,dh)=128, (v,D,E)=8]
    a1r = a1.reshape(8, 2, 2)  # [dh, v, E]
    W1 = (a0[:, None, None, :, None] * a1r[None, :, :, None, :])  # [a,dh,v,D,E]
    W1 = W1.reshape(128, 8)

    # G2 [de, e, abc] = sum_F a2[e,F] * core[(A,B,C),(D,E,F)]
    core_mat = core.astype(np.float64).reshape(8, 4, 2)  # [abc, de, F]
    G2 = np.einsum("eF,zdF->dez", a2, core_mat)  # [de, e, abc]

    # G2bd [(dlo,e)=128, (k=(v,de), group, abc) = 8*ROWS], nonzero iff
    # dlo == v; s2 is replicated into all 12 groups.
    G2bd6 = np.zeros((2, 64, 2, 4, ROWS // 8, 8))  # [dlo,e,v,de,grp,abc]
    for v in range(2):
        for g in range(ROWS // 8):
            G2bd6[v, :, v, :, g, :] = np.transpose(G2, (1, 0, 2))
    G2bd = G2bd6.reshape(128, 8 * ROWS)

    # W_out [abc=8, opq=32768]
    Wout = np.einsum(
        "oA,pB,qC->ABCopq",
        u0.astype(np.float64), u1.astype(np.float64), u2.astype(np.float64),
    ).reshape(8, FOUT)

    # 3-limb bf16 split of W_out. The limb operand rows are
    # [h x4 | m x4 | l x4]; pairing with scaled W groups makes every row
    # meaningful (scaling by 1/2 and 1/4 is exact in bf16):
    #   h rows: H/2 + M + L + H/2          = hH + hM + hL
    #   m rows: H/2 + M/2 + H/2 + M/2      = mH + mM
    #   l rows: H/4 x4                     = lH
    Wh = Wout.astype(BF)
    Wm = (Wout - Wh.astype(np.float64)).astype(BF)
    Wl = (Wout - Wh.astype(np.float64) - Wm.astype(np.float64)).astype(BF)
    Wh2 = (Wh.astype(np.float64) * 0.5).astype(BF)
    Wm2 = (Wm.astype(np.float64) * 0.5).astype(BF)
    Wh4 = (Wh.astype(np.float64) * 0.25).astype(BF)
    Wstack = np.concatenate(
        [Wh2, Wm, Wl, Wh2, Wh2, Wm2, Wh2, Wm2, Wh4, Wh4, Wh4, Wh4], axis=0
    )  # [96, FOUT]

    return (
        W1.astype(np.float32),
        G2bd.astype(np.float32),
        np.ascontiguousarray(Wstack),
    )


def _build(reps=1):
    nc = bacc.Bacc("TRN2", target_bir_lowering=False, debug=False)
    x_d = nc.dram_tensor("x", [S, FIN], F32, kind="ExternalInput")
    w1_d = nc.dram_tensor("w1", [128, 8], F32, kind="ExternalInput")
    g2_d = nc.dram_tensor("g2", [128, 8 * ROWS], F32, kind="ExternalInput")
    wl_d = nc.dram_tensor("wl", [ROWS, FOUT], BF16, kind="ExternalInput")
    y_d = nc.dram_tensor("y", [S, FOUT], F32, kind="ExternalOutput")

    with TileContext(nc) as tc:
        with (
            tc.tile_pool(name="consts", bufs=1) as cpool,
            tc.tile_pool(name="xp", bufs=4) as xp,
            tc.tile_pool(name="tp", bufs=2) as tp,
            tc.tile_pool(name="s2p", bufs=2) as s2p,
            tc.tile_pool(name="yp", bufs=3) as yp,
            tc.tile_pool(name="psA", bufs=2, space=bass.MemorySpace.PSUM) as psA,
            tc.tile_pool(name="psB", bufs=2, space=bass.MemorySpace.PSUM) as psB,
            tc.tile_pool(name="psC", bufs=3, space=bass.MemorySpace.PSUM) as psC,
        ):
            w1 = cpool.tile([128, 8], F32)
            nc.sync.dma_start(w1[:], w1_d[:])
            g2 = cpool.tile([128, 8 * ROWS], F32)
            nc.sync.dma_start(g2[:], g2_d[:])
            # W-limb stack (x loads ride the sync ring, wl + y stores the
            # scalar ring, so the streams don't serialize behind each other)
            wl = cpool.tile([ROWS, FOUT], BF16)
            nc.scalar.dma_start(wl[:], wl_d[:])

            def emit_block(blk, s2_ps):
                bw = blk % BLK_PER_WIN
                x_t = xp.tile([128, S_BLK * 128], F32, tag="x", name="x_t")
                src = x_d[blk * S_BLK:(blk + 1) * S_BLK, :].rearrange(
                    "s (p f) -> s p f", p=128
                ).transpose([1, 0, 2])
                nc.sync.dma_start(x_t[:], src)

                # stage A: one matmul per sample (x_s stationary)
                t_ps = psA.tile([128, S_BLK * 8], F32, tag="tps", name="t_ps")
                for sl in range(S_BLK):
                    nc.tensor.matmul(
                        t_ps[:, sl * 8:(sl + 1) * 8],
                        x_t[:, sl * 128:(sl + 1) * 128],
                        w1[:],
                        start=True, stop=True,
                    )
                t_sb = tp.tile([128, S_BLK * 8], F32, tag="tsb", name="t_sb")
                nc.vector.tensor_copy(t_sb[:], t_ps[:])

                # stage B: contract (dlo, e); accumulate all 8 (v,de)
                t_v = t_sb.rearrange("p (s k) -> p s k", k=8)
                for k in range(8):
                    nc.tensor.matmul(
                        s2_ps[:, bw * S_BLK:(bw + 1) * S_BLK],
                        g2[:, k * ROWS:(k + 1) * ROWS],
                        t_v[:, :, k],
                        start=(k == 0), stop=(k == 7),
                    )

            def emit_limb(s2_ps):
                # limb rows: h at 0..31, m at 32..63, l at 64..95
                limb = s2p.tile([ROWS, WIN], BF16, tag="limb", name="limb")
                hf = s2p.tile([ROWS, WIN], F32, tag="hf", name="hf")
                r1 = s2p.tile([ROWS, WIN], F32, tag="r1", name="r1")
                r2 = s2p.tile([ROWS, WIN], F32, tag="r2", name="r2")
                nc.vector.tensor_copy(limb[:], s2_ps[:])      # h everywhere
                nc.vector.tensor_copy(hf[:], limb[:])         # upcast h
                nc.vector.tensor_sub(r1[:], s2_ps[:], hf[:])  # r1 = s2 - h
                nc.vector.tensor_copy(limb[32:64, :], r1[32:64, :])   # m
                nc.vector.tensor_copy(limb[64:96, :], r1[64:96, :])   # m @ l rows
                nc.vector.tensor_copy(hf[64:96, :], limb[64:96, :])   # upcast m
                nc.vector.tensor_sub(r2[64:96, :], r1[64:96, :], hf[64:96, :])
                nc.vector.tensor_copy(limb[64:96, :], r2[64:96, :])   # l
                return limb

            def emit_ctile(w, st, limb):
                y_sb = yp.tile([128, YSTAGE], F32, tag="ysb", name="y_sb")
                for c8 in range(NC_PER_YSTAGE):
                    c = st * NC_PER_YSTAGE + c8
                    y_ps = psC.tile([128, YCHUNK], F32, tag="yps", name="y_ps")
                    nc.tensor.matmul(
                        y_ps[:], limb[:],
                        wl[:, c * YCHUNK:(c + 1) * YCHUNK],
                        start=True, stop=True,
                    )
                    dst = y_sb[:, c8 * YCHUNK:(c8 + 1) * YCHUNK]
                    if c8 % 2 == 0:
                        nc.vector.tensor_copy(dst, y_ps[:])
                    else:
                        nc.scalar.copy(dst, y_ps[:])
                nc.scalar.dma_start(
                    y_d[w * WIN:(w + 1) * WIN, st * YSTAGE:(st + 1) * YSTAGE],
                    y_sb[:],
                )

            for _rep in range(reps):
                # window 0 stages A/B
                s2_ps0 = psB.tile([ROWS, WIN], F32, tag="s2ps", name="s2_ps0")
                for bw in range(BLK_PER_WIN):
                    emit_block(bw, s2_ps0)
                limb0 = emit_limb(s2_ps0)
                # window 0 stage C interleaved with window 1 stages A/B
                s2_ps1 = psB.tile([ROWS, WIN], F32, tag="s2ps", name="s2_ps1")
                for st in range(N_YSTAGE):
                    emit_ctile(0, st, limb0)
                    if st < BLK_PER_WIN:
                        emit_block(BLK_PER_WIN + st, s2_ps1)
                limb1 = emit_limb(s2_ps1)
                for st in range(N_YSTAGE):
                    emit_ctile(1, st, limb1)
    nc.compile()
    return nc


_NC_CACHE = []


def _get_nc():
    if not _NC_CACHE:
        _NC_CACHE.append(_build())
    return _NC_CACHE[0]


def run(inputs, trace=False):
    x = np.ascontiguousarray(np.asarray(inputs["x"], dtype=np.float32))
    W1, G2bd, Wstack = _host_weights(
        np.asarray(inputs["core"]),
        np.asarray(inputs["u0"]), np.asarray(inputs["u1"]),
        np.asarray(inputs["u2"]),
        np.asarray(inputs["a0"]), np.asarray(inputs["a1"]),
        np.asarray(inputs["a2"]),
    )
    x_flat = x.reshape(S_TOT, FIN)
    nc = _get_nc()
    in_maps = []
    for i in range(NCORES):
        in_maps.append({
            "x": np.ascontiguousarray(x_flat[i * S:(i + 1) * S]),
            "w1": W1,
            "g2": G2bd,
            "wl": Wstack,
        })
    res = run_bass_kernel_spmd(
        nc, in_maps, core_ids=list(range(NCORES)), trace=trace,
    )
    y = np.concatenate([r["y"] for r in res.results], axis=0)
    y = y.reshape(4, 64, 8, 256, 128)
    return y, res


def kernel(**inputs) -> np.ndarray:
    y, _ = run(inputs, trace=False)
    return y
